# revision 10
# baseline (speedup 1.0000x reference)
"""Per-sample modulated Conv3D (B=4, CIN=COUT=16, T=16, H=W=128, K=3x3x3)
on 8 TRN2 NeuronCores.

Sharding: data-parallel over (batch, T-half) -> 8 shards, no cross-core
communication. Each core computes a [16, 8, 128, 128] output slab.

Mapping: banded im2col. One matmul column = a base output location
(hb, w) with h = 4*hb + dh; the M dim packs (dt in 2, dh in 4, co in 16)
= 128 outputs per column, the K dim packs (kt~ in 4, kh~ in 6, ci in 16)
= 384 contraction rows split over 3 SBUF chunk tiles; kw in 3 is handled
by free-dim offsets into w-padded rows. 9 accumulated fp32r matmuls of
[K=128, M=128, N=512] per PSUM tile.
"""

import math

import numpy as np

import concourse.bass as bass
import concourse.bacc as bacc
import concourse.mybir as mybir
from concourse.tile import TileContext
from concourse.bass_utils import run_bass_kernel_spmd
from concourse.vector_clock import ScopedClock

B, CIN, COUT = 4, 16, 16
T, H, W = 16, 128, 128
K = 3
SCALE = 1.0 / math.sqrt(CIN * K * K * K)

N_CORES = 8
TSH = T // 2          # output t-planes per core
TAU = TSH + 2         # input planes per core (with halo)
HP = H + 2            # padded h'
WP = W + 2            # padded w'
DT, DH = 2, 4         # M-dim banding
NT = H // DH // 4     # n-tiles per t_b: hb in 32 values, 4 per tile -> 8
FREE = (H // DH) * WP  # per-partition free size of a chunk tile (32*130)

_cache = {}


class _TC(TileContext):
    """TileContext whose exit drain splits sem waits across standalone
    wait instructions — this walrus build caps sync waits per instruction
    and the stock exit drain can exceed it."""

    def _drain_and_barrier(self, tick_clock, wait_clock):
        nc = self.nc
        drain_inst = nc.sync.drain()
        wait_clock.add_sem_waits(
            drain_inst.ins, ScopedClock({None: tick_clock.global_clock})
        )
        si = drain_inst.ins.sync_info
        waits = list(si.on_wait) if si is not None and si.on_wait else []
        if len(waits) > 1:
            si.on_wait = waits[:1]
            assert self.sems is not None
            by_num = {h.num: h for h in self.sems.allocated().values()}
            for w in waits[1:]:
                nc.sync.wait_ge(by_num[w.id], w.wait_value)
        nc.all_engine_barrier()
        assert self.sems is not None
        popped = nc._tile_sem_poison_stack.pop()
        assert popped is self._sem_poison
        nc.clear_and_free_semaphores(list(self.sems.allocated().values()))
        nc.all_engine_barrier()


def _build_program():
    f32 = mybir.dt.float32
    f32r = mybir.dt.float32r
    nc = bacc.Bacc("TRN2", target_bir_lowering=False, debug=False)
    # x layout: [hb, tau, kl, ci, w'] where input plane row h' = 4*hb + kl.
    x = nc.dram_tensor("x", [33, TAU, 4, CIN, WP], f32, kind="ExternalInput")
    wb = nc.dram_tensor("wb", [128, 9 * 128], f32, kind="ExternalInput")
    o = nc.dram_tensor("o", [COUT, TSH, H, W], f32, kind="ExternalOutput")

    with TileContext(nc) as tc:
        with (
            tc.tile_pool(name="wt", bufs=1) as wt_pool,
            tc.tile_pool(name="rhs", bufs=2) as rhs_pool,
            tc.tile_pool(name="ps", bufs=4, space="PSUM") as ps_pool,
            tc.tile_pool(name="st", bufs=4) as st_pool,
        ):
            wt = wt_pool.tile([128, 9 * 128], f32r)
            nc.sync.dma_start(out=wt[:], in_=wb[:].bitcast(f32r))

            for tbi in range(TSH // DT):
                tb = DT * tbi
                # Load the 4-plane window as 3 chunk tiles of 128 K-rows:
                # chunk c holds kh~ in {2c, 2c+1}; partition
                # p = kt~*32 + khl*16 + ci.
                chunks = []
                for c in range(3):
                    rhs_t = rhs_pool.tile([128, FREE], f32r, tag=f"ch{c}")
                    # chunk c holds kh~ = 2c + khl; source kl index is
                    # (2c + khl) % 4 with an hb offset of (2c + khl) // 4.
                    hb0 = (2 * c) // 4
                    kl0 = (2 * c) % 4
                    rhs3 = rhs_t.rearrange("p (hb w) -> p hb w", w=WP)
                    for kt in range(4):
                        src = x[
                            hb0 : hb0 + 32, tb + kt, kl0 : kl0 + 2, :, :
                        ].rearrange("hb kl ci w -> kl ci hb w").bitcast(f32r)
                        nc.sync.dma_start(
                            out=rhs3[kt * 32 : (kt + 1) * 32], in_=src
                        )
                    chunks.append(rhs_t)

                st = st_pool.tile([128, NT * 512], f32)
                for j in range(NT):
                    ps = ps_pool.tile([128, 512], f32)
                    n_mm = 0
                    for c in range(3):
                        rhs3 = chunks[c].rearrange("p (hb w) -> p hb w", w=WP)
                        for kw in range(3):
                            rhs_ap = rhs3[:, 4 * j : 4 * j + 4, kw : kw + W]
                            nc.tensor.matmul(
                                ps[:],
                                lhsT=wt[
                                    :, (c * 3 + kw) * 128 : (c * 3 + kw + 1) * 128
                                ],
                                rhs=rhs_ap,
                                start=(n_mm == 0),
                                stop=(n_mm == 8),
                            )
                            n_mm += 1
                    nc.vector.tensor_copy(
                        out=st[:, j * 512 : (j + 1) * 512], in_=ps[:]
                    )
                # st layout: [(dt dh co), (hb w)] over the whole t_b slab.
                o5 = o.rearrange("co t (hb dh) w -> co t hb dh w", dh=DH)
                for dt in range(DT):
                    for dh in range(DH):
                        dst = o5[:, tb + dt, :, dh, :]
                        src = st[
                            dt * 64 + dh * 16 : dt * 64 + dh * 16 + 16, :
                        ].rearrange("co (hb w) -> co hb w", w=W)
                        nc.sync.dma_start(out=dst, in_=src)
    nc.compile()
    return nc


def _prep_inputs(input, condition_feature, weight):
    """Host-side shard + weight packing. Returns in_maps for the 8 cores."""
    x_pad = np.zeros((B, CIN, T + 2, HP, WP), np.float32)
    x_pad[:, :, 1 : T + 1, 1 : H + 1, 1 : W + 1] = input

    # Per-sample modulated weights: [B, CO, CI, K, K, K]
    wmod = (weight[None] * SCALE * condition_feature).astype(np.float32)

    in_maps = []
    for core in range(N_CORES):
        b, half = divmod(core, 2)
        # Device layout [hb, tau, kl, ci, w'] with h' = 4*hb + kl (h'
        # padded 130 -> 132 so hb spans 33).
        xh = np.zeros((CIN, TAU, 132, WP), np.float32)
        xh[:, :, :HP] = x_pad[b, :, 8 * half : 8 * half + TAU]
        xs = np.ascontiguousarray(
            xh.reshape(CIN, TAU, 33, 4, WP).transpose(2, 1, 3, 0, 4)
        )
        wm = wmod[b]
        Wb = np.zeros((128, 3, 3, 128), np.float32)  # [K, c, kw, M]
        for c in range(3):
            for kw in range(3):
                for kt_t in range(4):
                    for khl in range(2):
                        kh_t = 2 * c + khl
                        for dt in range(DT):
                            kt = kt_t - dt
                            if not 0 <= kt < K:
                                continue
                            for dh in range(DH):
                                kh = kh_t - dh
                                if not 0 <= kh < K:
                                    continue
                                K0 = kt_t * 32 + khl * 16
                                M0 = dt * 64 + dh * 16
                                Wb[K0 : K0 + 16, c, kw, M0 : M0 + 16] = wm[
                                    :, :, kt, kh, kw
                                ].T
        in_maps.append({"x": xs, "wb": np.ascontiguousarray(Wb.reshape(128, 9 * 128))})
    return in_maps


def _run(inputs, trace=False, tmpdir=None):
    if "nc" not in _cache:
        _cache["nc"] = _build_program()
    nc = _cache["nc"]
    in_maps = _prep_inputs(**inputs)
    res = run_bass_kernel_spmd(
        nc,
        in_maps,
        core_ids=list(range(N_CORES)),
        trace=trace,
        tmpdir=tmpdir,
    )
    out = np.empty((B, COUT, T, H, W), np.float32)
    for core in range(N_CORES):
        b, half = divmod(core, 2)
        out[b, :, 8 * half : 8 * half + TSH] = res.results[core]["o"]
    return out, res


def kernel(**inputs) -> np.ndarray:
    out, _ = _run(inputs, trace=False)
    return out
